# revision 37
# baseline (speedup 1.0000x reference)
"""Trainium2 Bass kernel for nn_AttentionTD (3-block deformable attention TD).

Self-contained: hardcodes all shapes. Data-parallel over batch B=8 across the
8 NeuronCores; each core runs the full 3-block DAT stack for one batch element.

v3: fp8 rpe windows + DoubleRow bias/AV matmuls, f32r q/k, fp8 attention
probabilities, software-pipelined blocks: the (DVE-only) head chain of block
b+1 is interleaved between the attention (PE/ACT) groups of block b so no
engine's in-order queue stalls another. The rpe table is stored as two
pre-shifted packed copies (cols x0s..x0s+63 and x0s+1..x0s+64, 64-pitch rows)
so each DoubleRow rhs is a plain 3-dim [n, 2, 512] slice of the gathered
window.
"""

import sys

sys.path.insert(0, "/opt/trn_rl_repo")

import numpy as np

# ---------------- problem constants ----------------
B, C, H, W = 8, 128, 64, 64
NCH = 64          # channels per DAT block
NH, HC = 4, 16    # heads, head channels
KS = 4
HWS = H * W       # 4096
HK = WK = 16
NS = HK * WK      # 256 sample points
EPS = 1e-5
NBLK = 3
SHIFT = -2.5      # exp(logit + SHIFT) keeps fp8e4 P below the NaN zone
# rpe table geometry: [blk][h][x0s (64)][copy (2)][row (128)][col (64)] fp8
TSLICE = 2 * 128 * 64         # 16384 per x0s slice
THEAD = 64 * TSLICE           # per (blk,h)
TBLK = NH * THEAD
NTAB = NBLK * TBLK            # 12_582_912 < 2^24 (f32-exact indices)
BOFF = 128 * 64               # element offset of the B (x+1) copy in a slice

# odd-poly fit of Phi(x)-0.5 on [-4.2, 4.2]; |gelu err| <= 8.1e-4
GELU_C = (3.9835367417e-01, -6.5049081711e-02, 8.9258658527e-03,
          -8.3530905304e-04, 4.9286461944e-05, -1.6299346107e-06,
          2.2820646656e-08)

_CACHE = {}


def _build_graph():
    from concourse import bacc, mybir, tile
    from concourse.bass import IndirectOffsetOnAxis

    f32 = mybir.dt.float32
    f32r = mybir.dt.float32r
    bf16 = mybir.dt.bfloat16
    f8 = mybir.dt.float8e4
    i32 = mybir.dt.int32
    Alu = mybir.AluOpType
    Act = mybir.ActivationFunctionType
    DR = mybir.MatmulPerfMode.DoubleRow

    nc = bacc.Bacc("TRN2", target_bir_lowering=False, debug=False, num_devices=8)

    # ---- dram io ----
    xi1_d = nc.dram_tensor("xi1", [C, HWS], f32, kind="ExternalInput").ap()
    xi2_d = nc.dram_tensor("xi2", [C, HWS], f32, kind="ExternalInput").ap()
    kvT0_d = nc.dram_tensor("kvT0", [HWS, NCH], f32, kind="ExternalInput").ap()
    kvT1_d = nc.dram_tensor("kvT1", [HWS, NCH], f32, kind="ExternalInput").ap()
    wpf_d = nc.dram_tensor("wpf", [64, 3 * 128], bf16, kind="ExternalInput").ap()
    xq1_d = nc.dram_tensor("xq1", [64, HWS], bf16, kind="ExternalInput").ap()
    xq2_d = nc.dram_tensor("xq2", [64, HWS], bf16, kind="ExternalInput").ap()
    wpb_d = nc.dram_tensor("wpb", [65, 3 * 192], bf16, kind="ExternalInput").ap()
    cp_d = nc.dram_tensor("cp", [128, 590], f32, kind="ExternalInput").ap()
    cpb_d = nc.dram_tensor("cpb", [128, 320], bf16, kind="ExternalInput").ap()
    tab_d = nc.dram_tensor("rpetab", [NTAB, 1], f8, kind="ExternalInput").ap()
    o1_d = nc.dram_tensor("o1", [C, HWS], f32, kind="ExternalOutput").ap()
    o2_d = nc.dram_tensor("o2", [C, HWS], f32, kind="ExternalOutput").ap()

    with tile.TileContext(nc) as tc:
        import contextlib

        ctx = contextlib.ExitStack()
        with ctx:
            cpool = ctx.enter_context(tc.tile_pool(name="const", bufs=1))
            xpool = ctx.enter_context(tc.tile_pool(name="xdata", bufs=1))
            qfpool = ctx.enter_context(tc.tile_pool(name="qf", bufs=2))
            sb = ctx.enter_context(tc.tile_pool(name="work", bufs=2))
            sbs = ctx.enter_context(tc.tile_pool(name="small", bufs=2))
            wpool = ctx.enter_context(tc.tile_pool(name="wins", bufs=3))
            ppool = ctx.enter_context(tc.tile_pool(name="probs", bufs=1))
            qkps = ctx.enter_context(tc.tile_pool(name="qk", bufs=2, space="PSUM"))
            avps = ctx.enter_context(tc.tile_pool(name="av", bufs=1, space="PSUM"))
            mps = ctx.enter_context(tc.tile_pool(name="misc", bufs=1, space="PSUM"))

            # ---- persistent loads ----
            cp = cpool.tile([128, 590], f32, tag="cp")
            nc.sync.dma_start(out=cp[:, :], in_=cp_d)
            wpf = cpool.tile([64, 3 * 128], bf16, tag="wpf")
            nc.sync.dma_start(out=wpf[:, :], in_=wpf_d)
            xq1 = cpool.tile([64, HWS], bf16, tag="xq1")
            nc.sync.dma_start(out=xq1[:, :], in_=xq1_d)
            xq2 = cpool.tile([64, HWS], bf16, tag="xq2")
            nc.sync.dma_start(out=xq2[:, :], in_=xq2_d)
            wpb = cpool.tile([65, 3 * 192], bf16, tag="wpb")
            nc.sync.dma_start(out=wpb[:, :], in_=wpb_d)
            cpb = cpool.tile([128, 320], bf16, tag="cpb")
            nc.sync.dma_start(out=cpb[:, :], in_=cpb_d)
            xi1 = xpool.tile([C, HWS], f32, tag="xi1")
            nc.sync.dma_start(out=xi1[:, :], in_=xi1_d)
            xi2 = xpool.tile([C, HWS], f32, tag="xi2")
            nc.sync.dma_start(out=xi2[:, :], in_=xi2_d)

            def act_raw(out, in_, func):
                eng = nc.scalar
                ins = [eng.lower_ap(in_)]
                for v in (0.0, 1.0, 0.0):
                    ins.append(mybir.ImmediateValue(dtype=mybir.dt.float32, value=v))
                return eng.add_instruction(
                    mybir.InstActivation(
                        name=nc.get_next_instruction_name(), func=func,
                        ins=ins, outs=[eng.lower_ap(out)],
                    )
                )

            ebias = cpool.tile([128, 1], f32, tag="ebias")
            nc.vector.memset(ebias[:, :], SHIFT)
            c_shift1 = cpool.tile([1, 1], i32, tag="c_shift1")
            nc.vector.memset(c_shift1[:, :], 1)
            c_ones = cpool.tile([1, 1], i32, tag="c_ones")
            nc.vector.memset(c_ones[:, :], -1)
            c_magic = cpool.tile([1, 1], i32, tag="c_magic")
            nc.vector.memset(c_magic[:, :], 0x5F3759E0)  # magic + 1

            eye = cp[:, 0:128]
            ref_yx = cp[0:2, 128:384]          # row0 = y, row1 = x
            ones1_128 = cp[0:1, 384:512]       # [1,128] ones (bcast lhsT)
            ones128_div = cp[0:128, 520:521]   # 1/64 on data rows, 0 on gaps

            def wf(blk, lo, hi):
                return wpf[:, blk * 128 + lo : blk * 128 + hi]

            def wb(blk, lo, hi, rows=64):
                return wpb[0:rows, blk * 192 + lo : blk * 192 + hi]

            # ================= stage 1: q projection =================
            def emit_headQ(blk, XQ):
                st = {"blk": blk}
                q_f = qfpool.tile([128, HWS], f32r, tag="qf", name=f"qf{blk}")
                pq_b_sp = cp[:, 521 + blk : 522 + blk]
                for mc in range(8):
                    qp = mps.tile([128, 512], f32, tag="m", name=f"qp{blk}_{mc}")
                    nc.tensor.matmul(
                        out=qp[:, :], lhsT=wf(blk, 0, 128),
                        rhs=XQ[0:64, mc * 512 : (mc + 1) * 512],
                        start=True, stop=True,
                    )
                    nc.vector.tensor_scalar(
                        out=q_f[:, mc * 512 : (mc + 1) * 512], in0=qp[:, :],
                        scalar1=pq_b_sp, scalar2=None, op0=Alu.add,
                    )
                st["q_f"] = q_f
                return st

            # ========== stage 2: head chain (DVE + a few PE, no ACT/Pool) =========
            def emit_headC(blk, st, kvT_ap):
                """Generator: yields at chunk boundaries; ends with the kv stage."""
                q_f = st["q_f"]
                bc0 = 527 + blk * 21
                dw_w = cp[:, bc0 : bc0 + 16]
                dw_b = cp[:, bc0 + 16 : bc0 + 17]
                ln_g = cp[:, bc0 + 17 : bc0 + 18]
                ln_b = cp[:, bc0 + 18 : bc0 + 19]
                pw_wT = cp[:, bc0 + 19 : bc0 + 21]

                # ---------- depthwise 4x4 stride-4 conv ----------
                q5 = q_f[:, :].rearrange("p (hh a ww b) -> p hh a ww b", hh=16, a=4, ww=16, b=4)
                acc = sbs.tile([128, NS], f32, tag="dwacc", name=f"acc{blk}")
                nc.vector.tensor_scalar(
                    out=acc[:, :], in0=q5[:, :, 0, :, 0], scalar1=dw_w[:, 0:1],
                    scalar2=None, op0=Alu.mult,
                )
                for t in range(1, 8):
                    dy, dx = t // 4, t % 4
                    nc.vector.scalar_tensor_tensor(
                        out=acc[:, :], in0=q5[:, :, dy, :, dx],
                        scalar=dw_w[:, t : t + 1], in1=acc[:, :],
                        op0=Alu.mult, op1=Alu.add,
                    )
                yield
                for t in range(8, 16):
                    dy, dx = t // 4, t % 4
                    nc.vector.scalar_tensor_tensor(
                        out=acc[:, :], in0=q5[:, :, dy, :, dx],
                        scalar=dw_w[:, t : t + 1], in1=acc[:, :],
                        op0=Alu.mult, op1=Alu.add,
                    )
                nc.vector.tensor_scalar(
                    out=acc[:, :], in0=acc[:, :], scalar1=dw_b, scalar2=None, op0=Alu.add
                )
                yield
                # ---------- layernorm over channels ----------
                sq = sbs.tile([128, NS], f32, tag="sq", name=f"sq{blk}")
                nc.vector.tensor_tensor(out=sq[:, :], in0=acc[:, :], in1=acc[:, :], op=Alu.mult)
                mu_p = mps.tile([1, NS], f32, tag="m", name=f"mu{blk}")
                nc.tensor.matmul(out=mu_p[:, :], lhsT=ones128_div, rhs=acc[:, :], start=True, stop=True)
                e2_p = mps.tile([1, NS], f32, tag="m", name=f"e2{blk}")
                nc.tensor.matmul(out=e2_p[:, :], lhsT=ones128_div, rhs=sq[:, :], start=True, stop=True)
                stats = sbs.tile([1, 2 * NS], f32, tag="stats", name=f"stats{blk}")
                nc.vector.tensor_copy(out=stats[:, 0:NS], in_=mu_p[:, :])
                mu2 = sbs.tile([1, NS], f32, tag="mu2", name=f"mu2{blk}")
                nc.vector.tensor_tensor(out=mu2[:, :], in0=stats[:, 0:NS], in1=stats[:, 0:NS], op=Alu.mult)
                var = sbs.tile([1, NS], f32, tag="var", name=f"var{blk}")
                nc.vector.tensor_tensor(out=var[:, :], in0=e2_p[:, :], in1=mu2[:, :], op=Alu.subtract)
                # rstd = rsqrt(var+eps): bit-trick seed + 2 Newton iterations
                nc.vector.tensor_scalar(out=var[:, :], in0=var[:, :], scalar1=EPS, scalar2=None, op0=Alu.add)
                iv = sbs.tile([1, NS], i32, tag="iv", name=f"iv{blk}")
                nc.vector.tensor_scalar(out=iv[:, :], in0=var[:, :].bitcast(i32), scalar1=c_shift1[:, :], scalar2=None, op0=Alu.logical_shift_right)
                nc.vector.tensor_scalar(out=iv[:, :], in0=iv[:, :], scalar1=c_ones[:, :], scalar2=None, op0=Alu.bitwise_xor)
                nc.vector.tensor_tensor(out=iv[:, :], in0=iv[:, :], in1=c_magic[:, :].to_broadcast([1, NS]), op=Alu.add)
                yv = iv[:, :].bitcast(f32)
                tn = sbs.tile([1, NS], f32, tag="tn", name=f"tn{blk}")
                rst = stats[:, NS : 2 * NS]
                nc.vector.tensor_tensor(out=tn[:, :], in0=yv, in1=yv, op=Alu.mult)
                nc.vector.tensor_tensor(out=tn[:, :], in0=tn[:, :], in1=var[:, :], op=Alu.mult)
                nc.vector.tensor_scalar(out=tn[:, :], in0=tn[:, :], scalar1=-0.5, scalar2=1.5, op0=Alu.mult, op1=Alu.add)
                nc.vector.tensor_tensor(out=rst, in0=yv, in1=tn[:, :], op=Alu.mult)
                nc.vector.tensor_tensor(out=tn[:, :], in0=rst, in1=rst, op=Alu.mult)
                nc.vector.tensor_tensor(out=tn[:, :], in0=tn[:, :], in1=var[:, :], op=Alu.mult)
                nc.vector.tensor_scalar(out=tn[:, :], in0=tn[:, :], scalar1=-0.5, scalar2=1.5, op0=Alu.mult, op1=Alu.add)
                nc.vector.tensor_tensor(out=rst, in0=rst, in1=tn[:, :], op=Alu.mult)
                yield
                bc_p = mps.tile([128, 2 * NS], f32, tag="m", name=f"bc{blk}")
                nc.tensor.matmul(out=bc_p[:, :], lhsT=ones1_128, rhs=stats[:, :], start=True, stop=True)
                t1 = sbs.tile([128, NS], f32, tag="t1", name=f"t1{blk}")
                nc.vector.tensor_tensor(out=t1[:, :], in0=acc[:, :], in1=bc_p[:, 0:NS], op=Alu.subtract)
                nc.vector.tensor_tensor(out=t1[:, :], in0=t1[:, :], in1=bc_p[:, NS : 2 * NS], op=Alu.mult)
                nc.vector.tensor_scalar(
                    out=t1[:, :], in0=t1[:, :], scalar1=ln_g, scalar2=ln_b,
                    op0=Alu.mult, op1=Alu.add,
                )
                yield
                # ---------- GELU via odd-poly Phi (no ACT) ----------
                xc = sbs.tile([128, NS], f32, tag="xc", name=f"xc{blk}")
                nc.vector.tensor_scalar(out=xc[:, :], in0=t1[:, :], scalar1=4.2, scalar2=-4.2, op0=Alu.min, op1=Alu.max)
                uu = sbs.tile([128, NS], f32, tag="uu", name=f"uu{blk}")
                nc.vector.tensor_tensor(out=uu[:, :], in0=xc[:, :], in1=xc[:, :], op=Alu.mult)
                pp = sbs.tile([128, NS], f32, tag="pp", name=f"pp{blk}")
                nc.vector.tensor_scalar(out=pp[:, :], in0=uu[:, :], scalar1=GELU_C[6], scalar2=GELU_C[5], op0=Alu.mult, op1=Alu.add)
                for k in (4, 3, 2):
                    nc.vector.tensor_tensor(out=pp[:, :], in0=pp[:, :], in1=uu[:, :], op=Alu.mult)
                    nc.vector.tensor_scalar(out=pp[:, :], in0=pp[:, :], scalar1=GELU_C[k], scalar2=None, op0=Alu.add)
                yield
                for k in (1, 0):
                    nc.vector.tensor_tensor(out=pp[:, :], in0=pp[:, :], in1=uu[:, :], op=Alu.mult)
                    nc.vector.tensor_scalar(out=pp[:, :], in0=pp[:, :], scalar1=GELU_C[k], scalar2=None, op0=Alu.add)
                nc.vector.tensor_tensor(out=pp[:, :], in0=pp[:, :], in1=xc[:, :], op=Alu.mult)
                tf = sbs.tile([128, NS], f32, tag="tf", name=f"tf{blk}")
                nc.vector.tensor_tensor(out=tf[:, :], in0=t1[:, :], in1=pp[:, :], op=Alu.mult)
                gl = sbs.tile([128, NS], f32, tag="gl", name=f"gl{blk}")
                nc.vector.scalar_tensor_tensor(out=gl[:, :], in0=t1[:, :], scalar=0.5, in1=tf[:, :], op0=Alu.mult, op1=Alu.add)
                # ---------- offsets -> positions ----------
                off_p = mps.tile([2, NS], f32, tag="m", name=f"off{blk}")
                nc.tensor.matmul(out=off_p[:, :], lhsT=pw_wT, rhs=gl[:, :], start=True, stop=True)
                pos = sbs.tile([2, NS], f32, tag="pos", name=f"pos{blk}")
                nc.vector.tensor_tensor(out=pos[:, :], in0=off_p[:, :], in1=ref_yx, op=Alu.add)
                nc.vector.tensor_scalar(
                    out=pos[:, :], in0=pos[:, :], scalar1=1.0, scalar2=-1.0,
                    op0=Alu.min, op1=Alu.max,
                )
                # transpose pos -> [n,(y,x)] per 128-chunk
                posT = sbs.tile([128, 4], f32, tag="posT", name=f"posT{blk}")
                for c in range(2):
                    tp = mps.tile([128, 2], f32, tag="m", name=f"tp{blk}_{c}")
                    nc.tensor.transpose(
                        out=tp[:, :], in_=pos[:, c * 128 : (c + 1) * 128], identity=eye[0:2, 0:2]
                    )
                    nc.vector.tensor_copy(out=posT[:, c * 2 : c * 2 + 2], in_=tp[:, :])
                yield
                # ---------- index & weight math ([128,2] chunk pairs) ----------
                idxkv = sbs.tile([128, 8], f32, tag="idxkv", name=f"idxkv{blk}")
                idxw = sbs.tile([128, 8], f32, tag="idxw", name=f"idxw{blk}")
                wkv = sbs.tile([128, 8], f32, tag="wkv", name=f"wkv{blk}")
                wbi = sbs.tile([128, 8], f32, tag="wbi", name=f"wbi{blk}")
                scr = sbs.tile([128, 24], f32, tag="scr", name=f"scr{blk}")

                yy = posT[:, 0:4:2]
                xx = posT[:, 1:4:2]
                xf = scr[:, 0:2]
                yf = scr[:, 2:4]
                xm = scr[:, 4:6]
                ym = scr[:, 6:8]
                x0 = scr[:, 8:10]
                y0 = scr[:, 10:12]
                fx = scr[:, 12:14]
                fy = scr[:, 14:16]
                fx1 = scr[:, 16:18]
                fy1 = scr[:, 18:20]
                ib = scr[:, 20:22]
                nc.vector.tensor_scalar(out=xf, in0=xx, scalar1=1.0, scalar2=31.5, op0=Alu.add, op1=Alu.mult)
                nc.vector.tensor_scalar(out=yf, in0=yy, scalar1=1.0, scalar2=31.5, op0=Alu.add, op1=Alu.mult)
                nc.vector.tensor_scalar(out=x0, in0=xf, scalar1=8388608.0, scalar2=-8388608.0, op0=Alu.add, op1=Alu.add)
                nc.vector.tensor_tensor(out=xm, in0=x0, in1=xf, op=Alu.is_gt)
                nc.vector.tensor_tensor(out=x0, in0=x0, in1=xm, op=Alu.subtract)
                nc.vector.tensor_scalar(out=x0, in0=x0, scalar1=62.0, scalar2=None, op0=Alu.min)
                nc.vector.tensor_scalar(out=y0, in0=yf, scalar1=8388608.0, scalar2=-8388608.0, op0=Alu.add, op1=Alu.add)
                nc.vector.tensor_tensor(out=ym, in0=y0, in1=yf, op=Alu.is_gt)
                nc.vector.tensor_tensor(out=y0, in0=y0, in1=ym, op=Alu.subtract)
                nc.vector.tensor_scalar(out=y0, in0=y0, scalar1=62.0, scalar2=None, op0=Alu.min)
                nc.vector.tensor_tensor(out=fx, in0=xf, in1=x0, op=Alu.subtract)
                nc.vector.tensor_tensor(out=fy, in0=yf, in1=y0, op=Alu.subtract)
                nc.vector.tensor_scalar(out=fx1, in0=fx, scalar1=-1.0, scalar2=1.0, op0=Alu.mult, op1=Alu.add)
                nc.vector.tensor_scalar(out=fy1, in0=fy, scalar1=-1.0, scalar2=1.0, op0=Alu.mult, op1=Alu.add)
                nc.vector.tensor_tensor(out=wkv[:, 0:5:4], in0=fy1, in1=fx1, op=Alu.mult)
                nc.vector.tensor_tensor(out=wkv[:, 1:6:4], in0=fy1, in1=fx, op=Alu.mult)
                nc.vector.tensor_tensor(out=wkv[:, 2:7:4], in0=fy, in1=fx1, op=Alu.mult)
                nc.vector.tensor_tensor(out=wkv[:, 3:8:4], in0=fy, in1=fx, op=Alu.mult)
                nc.vector.scalar_tensor_tensor(out=ib, in0=y0, scalar=64.0, in1=x0, op0=Alu.mult, op1=Alu.add)
                for t, offt in enumerate((0.0, 1.0, 64.0, 65.0)):
                    nc.vector.tensor_scalar(
                        out=idxkv[:, t : t + 5 : 4], in0=ib,
                        scalar1=offt, scalar2=None, op0=Alu.add,
                    )
                idxkv_i = sbs.tile([128, 8], i32, tag="idxkvi", name=f"idxkvi{blk}")
                nc.vector.tensor_copy(out=idxkv_i[:, :], in_=idxkv[:, :])
                st["idxkv_i"] = idxkv_i
                yield
                cxf = scr[:, 0:2]
                cyf = scr[:, 2:4]
                nc.vector.tensor_scalar(out=cxf, in0=xx, scalar1=-31.5, scalar2=31.5, op0=Alu.mult, op1=Alu.add)
                nc.vector.tensor_scalar(out=cyf, in0=yy, scalar1=-31.5, scalar2=31.5, op0=Alu.mult, op1=Alu.add)
                fbx = scr[:, 4:6]
                fby = scr[:, 6:8]
                x0b = scr[:, 8:10]
                y0b = scr[:, 10:12]
                nc.vector.tensor_scalar(out=x0b, in0=cxf, scalar1=8388608.0, scalar2=-8388608.0, op0=Alu.add, op1=Alu.add)
                nc.vector.tensor_tensor(out=fbx, in0=x0b, in1=cxf, op=Alu.is_gt)
                nc.vector.tensor_tensor(out=x0b, in0=x0b, in1=fbx, op=Alu.subtract)
                nc.vector.tensor_scalar(out=y0b, in0=cyf, scalar1=8388608.0, scalar2=-8388608.0, op0=Alu.add, op1=Alu.add)
                nc.vector.tensor_tensor(out=fby, in0=y0b, in1=cyf, op=Alu.is_gt)
                nc.vector.tensor_tensor(out=y0b, in0=y0b, in1=fby, op=Alu.subtract)
                nc.vector.tensor_tensor(out=fbx, in0=cxf, in1=x0b, op=Alu.subtract)
                nc.vector.tensor_tensor(out=fby, in0=cyf, in1=y0b, op=Alu.subtract)
                fbx1 = scr[:, 12:14]
                fby1 = scr[:, 14:16]
                nc.vector.tensor_scalar(out=fbx1, in0=fbx, scalar1=-1.0, scalar2=1.0, op0=Alu.mult, op1=Alu.add)
                nc.vector.tensor_scalar(out=fby1, in0=fby, scalar1=-1.0, scalar2=1.0, op0=Alu.mult, op1=Alu.add)
                nc.vector.tensor_tensor(out=wbi[:, 0:5:4], in0=fby1, in1=fbx1, op=Alu.mult)
                nc.vector.tensor_tensor(out=wbi[:, 1:6:4], in0=fby1, in1=fbx, op=Alu.mult)
                nc.vector.tensor_tensor(out=wbi[:, 2:7:4], in0=fby, in1=fbx1, op=Alu.mult)
                nc.vector.tensor_tensor(out=wbi[:, 3:8:4], in0=fby, in1=fbx, op=Alu.mult)
                iw = scr[:, 20:22]
                nc.vector.scalar_tensor_tensor(out=iw, in0=x0b, scalar=256.0, in1=y0b, op0=Alu.mult, op1=Alu.add)
                nc.vector.tensor_scalar(
                    out=iw, in0=iw, scalar1=64.0, scalar2=float(blk * TBLK),
                    op0=Alu.mult, op1=Alu.add,
                )
                for hh in range(4):
                    nc.vector.tensor_scalar(
                        out=idxw[:, hh : hh + 5 : 4], in0=iw,
                        scalar1=float(hh * THEAD), scalar2=None, op0=Alu.add,
                    )
                idxw_i = sbs.tile([128, 8], i32, tag="idxwi", name=f"idxwi{blk}")
                nc.vector.tensor_copy(out=idxw_i[:, :], in_=idxw[:, :])
                st["idxw_i"] = idxw_i
                st["wkv"] = wkv
                yield
                # DoubleRow diag weights for the bias taps, per chunk.
                # Pairing is over the two y-taps (i-stride 64 in the window),
                # so ddA = (w00, w10) serves the A copy, ddB = (w01, w11) the B.
                dds = []
                for c in range(2):
                    dd0 = sbs.tile([128, 2, 128], f8, tag=f"dd0_{c}", name=f"dd0_{blk}_{c}")
                    dd1 = sbs.tile([128, 2, 128], f8, tag=f"dd1_{c}", name=f"dd1_{blk}_{c}")
                    for t in range(2):
                        nc.vector.tensor_scalar(out=dd0[:, t, :], in0=eye, scalar1=wbi[:, c * 4 + 2 * t : c * 4 + 2 * t + 1], scalar2=None, op0=Alu.mult)
                        nc.vector.tensor_scalar(out=dd1[:, t, :], in0=eye, scalar1=wbi[:, c * 4 + 2 * t + 1 : c * 4 + 2 * t + 2], scalar2=None, op0=Alu.mult)
                    dds.append((dd0, dd1))
                st["dds"] = dds
                yield
                emit_kv(blk, st, kvT_ap)

            # ========== stage 3: kv gather + k/v projections ==========
            def emit_kv(blk, st, kvT_ap):
                idxkv_i = st["idxkv_i"]
                wkv = st["wkv"]
                pk_wTs1 = wb(blk, 0, 128, rows=65)
                pv_wT1 = wb(blk, 128, 192, rows=65)
                G = sb.tile([128, 8, 64], f32, tag="G", name=f"G{blk}")
                for j in range(8):
                    nc.gpsimd.indirect_dma_start(
                        out=G[:, j, :], out_offset=None, in_=kvT_ap,
                        in_offset=IndirectOffsetOnAxis(ap=idxkv_i[:, j : j + 1], axis=0),
                    )
                xs_b = sb.tile([65, NS], bf16, tag="xsb", name=f"xsb{blk}")
                nc.vector.memset(xs_b[64:65, :], 1.0)
                for c in range(2):
                    xsT = sb.tile([128, 64], f32, tag="xsT", name=f"xsT{blk}_{c}")
                    nc.vector.tensor_scalar(
                        out=xsT[:, :], in0=G[:, c * 4 + 0, :],
                        scalar1=wkv[:, c * 4 : c * 4 + 1], scalar2=None, op0=Alu.mult,
                    )
                    for t in range(1, 4):
                        nc.vector.scalar_tensor_tensor(
                            out=xsT[:, :], in0=G[:, c * 4 + t, :],
                            scalar=wkv[:, c * 4 + t : c * 4 + t + 1], in1=xsT[:, :],
                            op0=Alu.mult, op1=Alu.add,
                        )
                    xs_p = mps.tile([64, 128], f32, tag="m", name=f"xsp{blk}_{c}")
                    nc.tensor.transpose(out=xs_p[:, :], in_=xsT[:, :], identity=eye)
                    nc.vector.tensor_copy(out=xs_b[0:64, c * 128 : (c + 1) * 128], in_=xs_p[:, :])

                k_p = mps.tile([128, NS], f32, tag="m", name=f"kp{blk}")
                nc.tensor.matmul(out=k_p[:, :], lhsT=pk_wTs1, rhs=xs_b[:, :], start=True, stop=True)
                k_b = sb.tile([128, NS], f32r, tag="kb", name=f"kb{blk}")
                nc.vector.tensor_copy(out=k_b[:, :], in_=k_p[:, :])
                st["k_b"] = k_b

                vT1 = sb.tile([128, 2, 128], f8, tag="vT1", name=f"vT1{blk}")
                nc.vector.memset(vT1[:, :, :], 0.0)
                nc.vector.memset(vT1[:, :, :].rearrange("p c (h q) -> p c h q", q=32)[:, :, :, 16:17], 1.0)
                for c in range(2):
                    v_p = mps.tile([128, 64], f32, tag="m", name=f"vp{blk}_{c}")
                    nc.tensor.matmul(
                        out=v_p[:, :], lhsT=xs_b[:, c * 128 : (c + 1) * 128], rhs=pv_wT1,
                        start=True, stop=True,
                    )
                    vv = vT1[:, c, :].rearrange("p (h q) -> p h q", q=32)
                    nc.vector.tensor_copy(
                        out=vv[:, :, 0:16],
                        in_=v_p[:, :].rearrange("p (h q) -> p h q", q=16),
                    )
                st["vT1"] = vT1

            # ========== stage 4: attention (PE + ACT heavy) ==========
            def emit_attn(blk, st, R, interleave=None):
                def tick():
                    if interleave is not None:
                        next(interleave, None)

                q_f = st["q_f"]
                k_b = st["k_b"]
                vT1 = st["vT1"]
                dds = st["dds"]
                idxw_i = st["idxw_i"]
                po_wT_sp = cpb[:, 128 + blk * 64 : 128 + (blk + 1) * 64]
                b4 = cpb[:, 0:128]
                po_b_hi = cp[64:128, 524 + blk : 525 + blk]

                avs = xpool.tile([128, HWS], bf16, tag="avs", name=f"avs{blk}")

                def drview(Wt, cp, mc):
                    # [p][i: stride 64 (y-tap)][512: stride 1] into copy cp
                    v = Wt[:, cp, mc * 512 : mc * 512 + 1].copy()
                    ap = v.ap
                    ap[-1] = [64, 2]
                    ap.append([1, 512])
                    return v

                ptiles = []
                for h in range(4):
                    # P layout [p, mc(8), c(2), 512] so AV's c-pairs are contiguous
                    P = ppool.tile([128, 8, 2, 512], f8, tag=f"P{h}", name=f"P{blk}_{h}")
                    ptiles.append(P)
                    for c in range(2):
                        Wt = wpool.tile([128, 2, 4160], f8, tag="W", name=f"W{blk}_{h}_{c}")
                        nc.gpsimd.indirect_dma_start(
                            out=Wt[:, 0, :], out_offset=None, in_=tab_d,
                            in_offset=IndirectOffsetOnAxis(ap=idxw_i[:, c * 4 + h : c * 4 + h + 1], axis=0),
                        )
                        nc.gpsimd.indirect_dma_start(
                            out=Wt[:, 1, :], out_offset=None, in_=tab_d,
                            in_offset=IndirectOffsetOnAxis(ap=idxw_i[:, c * 4 + h : c * 4 + h + 1], axis=0),
                            element_offset=BOFF,
                        )
                        ddA, ddB = dds[c]
                        kh = k_b[h * 32 : h * 32 + 16, c * 128 : (c + 1) * 128]
                        for wv in range(2):
                            pts = [
                                qkps.tile([128, 2, 512], f32, tag="qkp", name=f"pt{j}")
                                for j in range(2)
                            ]
                            for i in range(4):
                                mc = wv * 4 + i
                                nc.tensor.matmul(
                                    out=pts[i // 2][:, i % 2, :], lhsT=kh,
                                    rhs=q_f[h * 32 : h * 32 + 16, mc * 512 : (mc + 1) * 512],
                                    start=True, stop=False, tile_position=(h * 32, 0),
                                )
                            for i in range(4):
                                mc = wv * 4 + i
                                nc.tensor.matmul(
                                    out=pts[i // 2][:, i % 2, :], lhsT=ddA[:, :, :],
                                    rhs=drview(Wt, 0, mc),
                                    start=False, stop=False, perf_mode=DR,
                                )
                            for i in range(4):
                                mc = wv * 4 + i
                                nc.tensor.matmul(
                                    out=pts[i // 2][:, i % 2, :], lhsT=ddB[:, :, :],
                                    rhs=drview(Wt, 1, mc),
                                    start=False, stop=True, perf_mode=DR,
                                )
                            for j in range(2):
                                mc = wv * 4 + j * 2
                                nc.scalar.activation(
                                    out=P[:, mc : mc + 2, c, :],
                                    in_=pts[j][:, :, :].rearrange("p a b -> p (a b)"),
                                    func=Act.Exp, bias=ebias[:, :],
                                )
                        tick()
                # drain remaining interleave chunks (incl. next block's kv stage)
                if interleave is not None:
                    for _ in interleave:
                        pass
                    interleave = None
                # AV: one DoubleRow matmul per (h, mc), all at psum base 0
                for mc in range(8):
                    for h in range(4):
                        avp = avps.tile([32, 512], f32, tag=f"avp{h % 3}", bufs=1, name=f"avp{blk}_{mc}_{h}")
                        nc.tensor.matmul(
                            out=avp[:, :],
                            lhsT=vT1[:, :, h * 32 : (h + 1) * 32],
                            rhs=ptiles[h][:, mc, :, :],
                            start=True, stop=True, perf_mode=DR,
                        )
                        dst = avs[h * 32 : (h + 1) * 32, mc * 512 : (mc + 1) * 512]
                        if h < 2:
                            act_raw(dst, avp[:, :], Act.Copy)
                        else:
                            nc.vector.tensor_copy(out=dst, in_=avp[:, :])
                # ---------- normalize + out projection + residual ----------
                for mc in range(8):
                    sb_p = qkps.tile([128, 512], f32, tag="qkp", name=f"sbp{blk}_{mc}")
                    nc.tensor.matmul(out=sb_p[:, :], lhsT=b4, rhs=avs[:, mc * 512 : (mc + 1) * 512], start=True, stop=True)
                    rcp = sbs.tile([128, 512], f32, tag="rcp", name=f"rcp{blk}_{mc}")
                    nc.vector.reciprocal_approx_fast(out=rcp[:, :], in_=sb_p[:, :])
                    on = sbs.tile([128, 512], bf16, tag="on", name=f"on{blk}_{mc}")
                    nc.vector.tensor_tensor(out=on[:, :], in0=avs[:, mc * 512 : (mc + 1) * 512], in1=rcp[:, :], op=Alu.mult)
                    op = avps.tile([64, 512], f32, tag="avp2", bufs=1, name=f"op{blk}_{mc}")
                    nc.tensor.matmul(out=op[:, :], lhsT=po_wT_sp, rhs=on[:, :], start=True, stop=True)
                    nc.vector.scalar_tensor_tensor(
                        out=R[64:128, mc * 512 : (mc + 1) * 512], in0=op[:, :], scalar=po_b_hi,
                        in1=R[64:128, mc * 512 : (mc + 1) * 512], op0=Alu.add, op1=Alu.add,
                    )

            # ================= pipelined emission =================
            st0 = emit_headQ(0, xq1)
            for _ in emit_headC(0, st0, kvT0_d):
                pass

            st1 = emit_headQ(1, xq2)
            emit_attn(0, st0, xi1, interleave=emit_headC(1, st1, kvT0_d))

            st2 = emit_headQ(2, xq2)
            emit_attn(1, st1, xi2, interleave=emit_headC(2, st2, kvT1_d))

            emit_attn(2, st2, xi2, interleave=None)

            nc.sync.dma_start(out=o1_d, in_=xi1[:, :])
            nc.sync.dma_start(out=o2_d, in_=xi2[:, :])

    nc.compile()
    return nc


def _host_prep(inputs):
    """Build per-core in_maps. inputs: dict of full numpy arrays."""
    import ml_dtypes

    x0, x1, x2 = inputs["x0"], inputs["x1"], inputs["x2"]

    def spread_cols(m):
        # m: [64(in), 64(out)] -> [64(in), 128] with out col h*16+j at h*32+j
        out = np.zeros((m.shape[0], 128), m.dtype)
        for h in range(4):
            out[:, h * 32 : h * 32 + 16] = m[:, h * 16 : (h + 1) * 16]
        return out

    def spread_rows(v):
        # v: [64, k] -> [128, k] with row h*16+j at h*32+j
        out = np.zeros((128,) + v.shape[1:], v.dtype)
        for h in range(4):
            out[h * 32 : h * 32 + 16] = v[h * 16 : (h + 1) * 16]
        return out

    # weight pack bf16: [64, 3*128]  (spread pq_wT)
    wpf = np.zeros((64, 3 * 128), ml_dtypes.bfloat16)
    for b in range(3):
        wpf[:, b * 128 : (b + 1) * 128] = spread_cols(inputs["pq_w"][b].T).astype(
            ml_dtypes.bfloat16
        )
    wpb = np.zeros((65, 3 * 192), ml_dtypes.bfloat16)
    for b in range(3):
        o = b * 192
        pk = np.zeros((65, 128), np.float32)
        pk[0:64] = spread_cols(inputs["pk_w"][b].T * 0.25)
        for h in range(4):
            pk[64, h * 32 : h * 32 + 16] = inputs["pk_b"][b][h * 16 : (h + 1) * 16] * 0.25
        wpb[:, o : o + 128] = pk.astype(ml_dtypes.bfloat16)
        wpb[:64, o + 128 : o + 192] = inputs["pv_w"][b].T.astype(ml_dtypes.bfloat16)
        wpb[64, o + 128 : o + 192] = inputs["pv_b"][b].astype(ml_dtypes.bfloat16)
    # const pack [128, 590]
    cp = np.zeros((128, 590), np.float32)
    cp[:, 0:128] = np.eye(128, dtype=np.float32)
    ys = (np.linspace(0.5, HK - 0.5, HK) / (HK - 1.0)) * 2.0 - 1.0
    cp[0, 128:384] = np.repeat(ys, WK)         # y per n (i-major)
    cp[1, 128:384] = np.tile(ys, HK)           # x per n
    cp[0, 384:512] = 1.0                       # ones1_128
    for h in range(4):
        cp[h * 32 : h * 32 + 16, 520] = 1.0 / 64.0
    for b in range(3):
        cp[:, 521 + b] = spread_rows(inputs["pq_b"][b][:, None])[:, 0]
        cp[64:128, 524 + b] = inputs["po_b"][b]
        bc0 = 527 + b * 21
        cp[:, bc0 : bc0 + 16] = spread_rows(inputs["dw_w"][b].reshape(64, 16))
        cp[:, bc0 + 16] = spread_rows(inputs["dw_b"][b][:, None])[:, 0]
        cp[:, bc0 + 17] = spread_rows(inputs["ln_g"][b][:, None])[:, 0]
        cp[:, bc0 + 18] = spread_rows(inputs["ln_b"][b][:, None])[:, 0]
        cp[:, bc0 + 19 : bc0 + 21] = spread_rows(inputs["pw_w"][b].T)
    cpb = np.zeros((128, 320), ml_dtypes.bfloat16)
    b4 = np.zeros((128, 128), np.float32)
    for h in range(4):
        b4[h * 32 + 16, h * 32 : (h + 1) * 32] = 1.0
    cpb[:, 0:128] = b4.astype(ml_dtypes.bfloat16)
    for b in range(3):
        poT = inputs["po_w"][b].T  # [c, o]
        for h in range(4):
            cpb[h * 32 : h * 32 + 16, 128 + b * 64 : 128 + (b + 1) * 64] = poT[
                h * 16 : (h + 1) * 16
            ].astype(ml_dtypes.bfloat16)
    # rpe window tables fp8: per (b,h,x0s): [A: rows x cols x0s..x0s+63][B: +1]
    tab = np.zeros((NBLK, NH, 64, 2, 128, 64), ml_dtypes.float8_e4m3)
    rpe = inputs["rpe"]
    for b in range(3):
        for h in range(4):
            pad = np.zeros((128, 129), np.float32)
            pad[0:127, 0:127] = rpe[b, h]
            for x0s in range(64):
                tab[b, h, x0s, 0] = pad[:, x0s : x0s + 64].astype(ml_dtypes.float8_e4m3)
                tab[b, h, x0s, 1] = pad[:, x0s + 1 : x0s + 65].astype(ml_dtypes.float8_e4m3)
    tab = tab.reshape(-1, 1)

    in_maps = []
    for bb in range(B):
        m = {
            "xi1": np.ascontiguousarray(x1[bb].reshape(C, HWS)),
            "xi2": np.ascontiguousarray(x2[bb].reshape(C, HWS)),
            "kvT0": np.ascontiguousarray(x0[bb, :64].reshape(64, HWS).T),
            "kvT1": np.ascontiguousarray(x1[bb, :64].reshape(64, HWS).T),
            "xq1": x1[bb, :64].reshape(64, HWS).astype(ml_dtypes.bfloat16),
            "xq2": x2[bb, :64].reshape(64, HWS).astype(ml_dtypes.bfloat16),
            "wpf": wpf,
            "wpb": wpb,
            "cp": cp,
            "cpb": cpb,
            "rpetab": tab,
        }
        in_maps.append(m)
    return in_maps


def kernel(**inputs):
    from concourse.bass_utils import run_bass_kernel_spmd

    if "nc" not in _CACHE:
        _CACHE["nc"] = _build_graph()
    nc = _CACHE["nc"]
    in_maps = _host_prep(inputs)
    res = run_bass_kernel_spmd(nc, in_maps, core_ids=list(range(8)))
    out = np.zeros((NBLK, B, C, H, W), np.float32)
    out[0] = inputs["x0"]
    for bb in range(B):
        out[1, bb] = res.results[bb]["o1"].reshape(C, H, W)
        out[2, bb] = res.results[bb]["o2"].reshape(C, H, W)
    return out
